# revision 33
# baseline (speedup 1.0000x reference)
"""Butterfly-Conv2d (nn_BConv2d) Trainium2 kernel, v6.

Math (reference): x(B=64,IC=16,32,32) -> y=x.reshape(IC,B,N=1024)[:,:,bitrev];
broadcast over OC=32; 10 radix-2 butterfly layers with per-(ic,oc) twiddles;
mean over ic; + bias -> (B,OC,32,32).

v6 (vs v5 ~121us):
  * DMA issue de-serialization: each dma_start occupies its issuing engine
    queue ~0.6us regardless of size (SWDGE descriptor generation), and v5
    serialized 118 of them on the Sync queue (~72us).  v6 merges everything
    into 32 weight-blob transfers (sync queue) + 16 y/coef transfers
    (gpsimd queue) + 1 misc + 4 outs (scalar queue).
  * Mixed-precision O slots: per-slot damage scores (quant-delta norm x
    downstream gain) are unioned across the 8 cores (the SPMD program is
    shared, so the fp8/bf16 position mask must be uniform; the row scales
    live in the data and differ per core).  The worst ~30% stay bf16, the
    rest go fp8 e4m3.  E slots are always fp8 (harmless).  Weight bytes:
    25.2MB (v4) -> 20.5MB (v5) -> ~14.5MB.
  * Per (og,ic) weights pack into one ragged bf16 blob: [E fp8 | O fp8 |
    O bf16], column offsets baked into the program; fp8 ranges read via
    AP.bitcast.  y/r8/v01/r8-nodup pack per ic into one [P,736] blob.
  * Optional GPSIMD offload (GPS_T8): one oc-half of the t8 mul per
    iteration runs on GpSimd (idle otherwise), shaving ~270ns/iter off the
    DVE critical chain.  Uses a non-dup r8 copy so every operand AP is <=4D.

Device layout: per ic tile [128, 512]: partition p = n & 127, free col =
C*64 + b with chunk C = n >> 7 (bits n9n8n7).
"""

import numpy as np
import ml_dtypes

B, IC, OC, H, W = 64, 16, 32, 32, 32
N = H * W          # 1024
NCORES = 8
OCL = OC // NCORES  # 4 oc per core
NOG = OCL // 2     # 2 og groups of 2 oc
NCH = 8            # chunks (n9n8n7)
P = 128            # partitions (n6..n0)
BF = ml_dtypes.bfloat16
F8 = ml_dtypes.float8_e4m3  # TRN FP8_EXP4: max finite 240
DUP = 2             # coefficient duplication for packed DVE reads
F8MAX = 240.0
FRAC_BF16 = 0.10    # fraction of O slots kept in bf16 (worst union damage)
                    # (emulated: 0.10 -> 6.3e-3, 0.15 -> 5.7e-3, gate 2e-2)
GPS_T8 = False      # offload one oc-half of the t8 mul to GpSimd
                    # (measured: GpSimd TTs contend for the shared SBUF port
                    # and slow DVE TTs ~24% -- net loss; keep False)

_EPS = np.float32(1e-20)

# ycf blob layout (bf16 units per partition)
YCF_Y, YCF_R8, YCF_V01, YCF_R8ND = 0, 512, 576, 704
YCF_LEN = 736


def _bitrev(n):
    bits = int(np.log2(n))
    idx = np.arange(n, dtype=np.int64)
    rev = np.zeros(n, dtype=np.int64)
    for b in range(bits):
        rev = (rev << 1) | ((idx >> b) & 1)
    return rev


def _compose_A7(tw):
    """Compose butterfly layers 0..6 into A[ic,oc,C,128,128] (C=8 chunks)."""
    ic, oc = tw.shape[0], tw.shape[1]
    A = np.zeros((ic, oc, NCH, P, P), dtype=np.float32)
    A[:] = np.eye(P, dtype=np.float32)
    for l in range(7):
        s = 1 << l
        nb_loc = P // (2 * s)
        t = tw[:, :, l].reshape(ic, oc, N // (2 * s), s, 2, 2)
        t = t.reshape(ic, oc, NCH, nb_loc, s, 2, 2)
        Av = A.reshape(ic, oc, NCH, nb_loc, 2, s, P)
        a0 = Av[:, :, :, :, 0]
        a1 = Av[:, :, :, :, 1]
        new0 = t[..., 0, 0, None] * a0 + t[..., 0, 1, None] * a1
        new1 = t[..., 1, 0, None] * a0 + t[..., 1, 1, None] * a1
        Av[:, :, :, :, 0] = new0
        Av[:, :, :, :, 1] = new1
    return A


def _safe(c):
    return np.where(np.abs(c) < _EPS, np.where(c < 0, -_EPS, _EPS), c)


def _stageB_coeffs(tw):
    """Ratio-trick coefficients for layers 7,8,9: each [ic,oc,P,NCH] f32."""
    ic, oc = tw.shape[0], tw.shape[1]
    pr = np.arange(P)
    cp = np.arange(NCH)

    t7 = tw[:, :, 7].reshape(ic, oc, 4, P, 2, 2)      # [k, p, po, q]
    k7, po7 = cp >> 1, cp & 1
    c0 = t7[:, :, k7[None, :], pr[:, None], po7[None, :], 0]
    c1 = t7[:, :, k7[None, :], pr[:, None], po7[None, :], 1]
    r7 = (c1 / _safe(c0)).astype(np.float32)
    pend7 = c0

    t8 = tw[:, :, 8].reshape(ic, oc, 2, 256, 2, 2)    # [k9, n7*128+p, po, q]
    k8, po8, n78 = cp >> 2, (cp >> 1) & 1, cp & 1
    s8 = np.stack([4 * k8 + 0 + n78, 4 * k8 + 2 + n78])
    u0 = t8[:, :, k8[None, :], n78[None, :] * P + pr[:, None], po8[None, :], 0]
    u1 = t8[:, :, k8[None, :], n78[None, :] * P + pr[:, None], po8[None, :], 1]
    e0 = u0 * pend7[:, :, :, s8[0]]
    e1 = u1 * pend7[:, :, :, s8[1]]
    r8 = (e1 / _safe(e0)).astype(np.float32)
    pend8 = e0

    t9 = tw[:, :, 9].reshape(ic, oc, 512, 2, 2)       # [m*128+p, po, q]
    po9, m9 = cp >> 2, cp & 3
    s9 = np.stack([m9, 4 + m9])
    w0 = t9[:, :, m9[None, :] * P + pr[:, None], po9[None, :], 0]
    w1 = t9[:, :, m9[None, :] * P + pr[:, None], po9[None, :], 1]
    v0 = (w0 * pend8[:, :, :, s9[0]] / IC).astype(np.float32)
    v1 = (w1 * pend8[:, :, :, s9[1]] / IC).astype(np.float32)

    return r7, r8, v0, v1


# static source maps (twiddle independent)
_CP = np.arange(NCH)
S8 = np.stack([4 * (_CP >> 2) + (_CP & 1), 4 * (_CP >> 2) + 2 + (_CP & 1)])
S9 = np.stack([_CP & 3, 4 + (_CP & 3)])
_G0 = (S8[0] >> 1).astype(np.int64)
_G1 = (S8[1] >> 1).astype(np.int64)


def _f8(x):
    return np.clip(
        np.clip(x, -F8MAX, F8MAX).astype(F8).astype(np.float32), -F8MAX, F8MAX
    )


def _slot_scores(lhsT_E, O, r8, v0, v1):
    """Union-over-cores damage score of e4m3-quantizing each O slot.

    score[ic,oc,k,po] ~ max_m ||quant_delta[:,m]||_2 * gain_zc[m, 2k+po]
    where gain_zc bounds |d out / d zc| through the L8/L9 coefficient paths.
    Returns mask[NOG, IC, 2, 4, 2] (True = keep bf16), uniform across cores.
    """
    Emax = np.abs(lhsT_E).max(axis=3)                  # [ic,oc,k,m]
    Omax = np.abs(O).max(axis=4)                       # [ic,oc,k,po,m]
    gain_y8p = np.zeros((IC, OC, P, NCH), np.float32)
    vs = (v0, v1)
    for e in range(2):
        for C in range(NCH):
            gain_y8p[:, :, :, S9[e][C]] += np.abs(vs[e][:, :, :, C])
    gain_zc = np.zeros((IC, OC, P, NCH), np.float32)
    for Cpp in range(NCH):
        gain_zc[:, :, :, S8[0][Cpp]] += gain_y8p[:, :, :, Cpp]
        gain_zc[:, :, :, S8[1][Cpp]] += gain_y8p[:, :, :, Cpp] * np.abs(
            r8[:, :, :, Cpp]
        )
    score = np.zeros((IC, OC, 4, 2), np.float32)
    for k in range(4):
        for po in range(2):
            Ss = F8MAX / np.maximum(
                np.maximum(Emax[:, :, k], Omax[:, :, k, po]), _EPS
            )
            v = O[:, :, k, po] * Ss[:, :, None, :]
            delta = (_f8(v) - v) / Ss[:, :, None, :]
            dn = np.linalg.norm(delta, axis=2)          # [ic,oc,m]
            score[:, :, k, po] = (dn * gain_zc[:, :, :, 2 * k + po]).max(axis=2)
    # positions: oc = core*OCL + 2*og + oo
    sc = score.reshape(IC, NCORES, NOG, 2, 4, 2).transpose(1, 2, 0, 3, 4, 5)
    vote = sc.max(axis=0)                               # [og, ic, oo, k, po]
    thr = np.quantile(vote, 1.0 - FRAC_BF16)
    mask = vote >= thr
    # cap bf16 slots per (og,ic) granule at 5 so granule sizes (and hence
    # per-iteration DMA arrival times) stay roughly uniform
    for og in range(NOG):
        for ic in range(IC):
            g = mask[og, ic]
            n = int(g.sum())
            if n > 5:
                v = vote[og, ic].ravel()
                keep = np.argsort(v)[::-1][:5]
                g2 = np.zeros(16, bool)
                g2[keep] = True
                mask[og, ic] = g2.reshape(2, 4, 2)
    # force the first granule all-fp8 so the first weight transfer is small
    # and the pipeline starts earlier
    mask[0, 0] = False
    return mask


def _prep_host(x, twiddle, bias):
    perm = _bitrev(N)
    y = np.ascontiguousarray(x).reshape(IC, B, N)[:, :, perm]
    y_dev = np.ascontiguousarray(
        y.reshape(IC, B, NCH, P).transpose(0, 3, 2, 1)
    ).reshape(IC, P, NCH * B).astype(BF)

    tw = np.asarray(twiddle, dtype=np.float32)
    A = _compose_A7(tw)                    # [IC, OC, NCH, P(out), P(in)]
    r7, r8, v0, v1 = _stageB_coeffs(tw)    # each [IC, OC, P, NCH]

    lhsT_E = A[:, :, 0::2].transpose(0, 1, 2, 4, 3)          # [ic,oc,4,p_in,m]
    lhsT_O = A[:, :, 1::2].transpose(0, 1, 2, 4, 3)
    r7m = r7.transpose(0, 1, 3, 2)                            # [ic,oc,C',m]
    O = lhsT_O[:, :, :, None] * r7m.reshape(IC, OC, 4, 2, 1, P)  # [ic,oc,k,po,p,m]

    mask16 = _slot_scores(lhsT_E, O, r8, v0, v1)   # [og,ic,oo,k,po] True=bf16

    # ragged blob layout, shared across cores: per (og,ic):
    #   [E: 512 u] [fp8 O slots: 64 u each] [bf16 O slots: 128 u each]
    # (u = bf16 unit = 2 bytes)
    offs = np.zeros((NOG, IC), np.int64)
    lens = np.zeros((NOG, IC), np.int64)
    cur = 0
    slot_pos = {}
    for og in range(NOG):
        for ic in range(IC):
            offs[og, ic] = cur
            n8 = int((~mask16[og, ic]).sum())
            i8 = i16 = 0
            for oo in range(2):
                for k in range(4):
                    for po in range(2):
                        if mask16[og, ic, oo, k, po]:
                            slot_pos[(og, ic, oo, k, po)] = (False, 512 + n8 * 64 + i16 * 128)
                            i16 += 1
                        else:
                            slot_pos[(og, ic, oo, k, po)] = (True, 512 + i8 * 64)
                            i8 += 1
            lens[og, ic] = 512 + n8 * 64 + i16 * 128
            cur += lens[og, ic]
    WTOT = int(cur)
    layout = {"offs": offs, "lens": lens, "slot_pos": slot_pos, "wtot": WTOT}

    bias_np = np.asarray(bias, dtype=np.float32).reshape(OC, NCH, P)

    in_maps, emu_maps = [], []
    for core in range(NCORES):
        osl = slice(core * OCL, (core + 1) * OCL)
        Eg = lhsT_E[:, osl]                     # [ic,ocl,k,p,m]
        Og = O[:, osl]                          # [ic,ocl,k,po,p,m]
        Emax = np.abs(Eg).max(axis=3)           # [ic,ocl,k,m]
        Omax = np.abs(Og).max(axis=4)           # [ic,ocl,k,po,m]
        # group scale covers E plus any fp8 O slot of the group
        m_oc = np.zeros((IC, OCL, 4, 2), bool)  # True = bf16
        for og in range(NOG):
            for oo in range(2):
                m_oc[:, 2 * og + oo] = mask16[og, :, oo]
        incl = np.where(m_oc.transpose(0, 1, 2, 3)[:, :, :, :, None], 0.0, Omax)
        gm = np.maximum(Emax, incl.max(axis=3))
        S = (F8MAX / np.maximum(gm, _EPS)).astype(np.float32)  # [ic,ocl,k,m]

        wEq = _f8(Eg * S[:, :, :, None, :])
        wOs = Og * S[:, :, :, None, None, :]
        for k in range(4):
            for po in range(2):
                sel = ~m_oc[:, :, k, po]
                wq = _f8(wOs[:, :, k, po])
                wOs[:, :, k, po] = np.where(sel[:, :, None, None], wq, wOs[:, :, k, po])

        Sp = S.transpose(0, 1, 3, 2)            # [ic,ocl,p,k]
        r8f = r8[:, osl] * (Sp[:, :, :, _G0] / Sp[:, :, :, _G1])
        v0f = v0[:, osl] / Sp[:, :, :, _G0[S9[0]]]
        v1f = v1[:, osl] / Sp[:, :, :, _G0[S9[1]]]

        # pack w blob
        wb = np.zeros((P, WTOT), dtype=BF)
        wb_u8 = wb.view(np.uint8).reshape(P, WTOT * 2)
        for og in range(NOG):
            for ic in range(IC):
                base = int(offs[og, ic])
                # E: [oo,k] fp8, 128 cols each -> 512 units
                eb = np.ascontiguousarray(
                    wEq[ic, 2 * og : 2 * og + 2].transpose(2, 0, 1, 3)
                ).reshape(P, 1024).astype(F8)
                wb_u8[:, base * 2 : base * 2 + 1024] = eb.view(np.uint8)
                for oo in range(2):
                    for k in range(4):
                        for po in range(2):
                            isf8, pos = slot_pos[(og, ic, oo, k, po)]
                            sl = np.ascontiguousarray(wOs[ic, 2 * og + oo, k, po])
                            bo = (base + pos) * 2
                            if isf8:
                                wb_u8[:, bo : bo + 128] = sl.astype(F8).view(np.uint8)
                            else:
                                wb_u8[:, bo : bo + 256] = sl.astype(BF).view(np.uint8)

        # ycf blob: y 512 | r8 dup 64 | v01 dup 128 | r8 nodup 32 (pad 736)
        ycf = np.zeros((IC, P, YCF_LEN), dtype=BF)
        ycf[:, :, YCF_Y:YCF_R8] = y_dev
        r8d = np.repeat(
            r8f.transpose(0, 2, 1, 3)[..., None], DUP, axis=-1
        ).reshape(IC, P, OCL * NCH * DUP)
        ycf[:, :, YCF_R8:YCF_V01] = r8d.astype(BF)
        v01d = np.repeat(
            np.stack([v0f, v1f], axis=2).transpose(0, 3, 1, 2, 4)[..., None],
            DUP, axis=-1,
        ).reshape(IC, P, OCL * 2 * NCH * DUP)
        ycf[:, :, YCF_V01:YCF_R8ND] = v01d.astype(BF)
        r8nd = r8f.transpose(0, 2, 1, 3).reshape(IC, P, OCL * NCH)
        ycf[:, :, YCF_R8ND : YCF_R8ND + OCL * NCH] = r8nd.astype(BF)

        # misc blob f32 [P, 96]: bias 32 f32 | ident 128 bf16 (64 f32 units)
        misc = np.zeros((P, 96), dtype=np.float32)
        misc[:, :32] = np.ascontiguousarray(
            bias_np[osl].transpose(2, 0, 1)
        ).reshape(P, OCL * NCH)
        ident = np.eye(P, dtype=np.float32).astype(BF)
        misc.view(np.uint8)[:, 128:384] = ident.view(np.uint8)

        in_maps.append({"w": wb, "ycf": ycf, "misc": misc})
        emu_maps.append(
            {
                "y": y_dev,
                "wE": wEq,      # f32 values (exactly fp8)
                "wO": wOs,      # f32 values (fp8 or bf16-roundable)
                "r8": r8f,
                "v01": (v0f, v1f),
                "bias": misc[:, :32],
            }
        )
    return in_maps, emu_maps, layout


def _emulate_core(em):
    """Numpy emulation of the device program (exact op/rounding semantics)."""
    y = em["y"].reshape(IC, P, NCH, B)
    wE, wO = em["wE"], em["wO"]
    r8a = em["r8"]
    v0a, v1a = em["v01"]
    bias = em["bias"].reshape(P, OCL, NCH)
    out = np.zeros((OCL, P, NCH, B), dtype=np.float32)
    for o in range(OCL):
        acc = np.zeros((P, NCH, B), dtype=np.float32)
        for ic in range(IC):
            yv = y[ic].astype(np.float32)
            z = np.zeros((P, NCH, B), dtype=np.float32)
            for Cp in range(NCH):
                k, po = Cp >> 1, Cp & 1
                lE = wE[ic, o, k]
                lO = wO[ic, o, k, po].astype(BF).astype(np.float32)
                z[:, Cp] = lE.T @ yv[:, 2 * k] + lO.T @ yv[:, 2 * k + 1]
            zc = z.astype(BF).astype(np.float32)
            r8 = r8a[ic, o].astype(BF).astype(np.float32)
            v0 = v0a[ic, o].astype(BF).astype(np.float32)
            v1 = v1a[ic, o].astype(BF).astype(np.float32)
            tmp8 = (r8[:, :, None] * zc[:, S8[1]]).astype(BF).astype(np.float32)
            y8p = (zc[:, S8[0]] + tmp8).astype(BF).astype(np.float32)
            t9a = (v0[:, :, None] * y8p[:, S9[0]]).astype(BF).astype(np.float32)
            t9b = (v1[:, :, None] * y8p[:, S9[1]]).astype(BF).astype(np.float32)
            acc += t9a + t9b
        out[o] = acc + bias[:, o, :, None]
    return out.reshape(OCL, P, NCH * B)


_LAST_RESULTS = {"exec_time_ns": None}


def kernel(x, twiddle, bias, _trace=False, _emulate=False):
    in_maps, emu_maps, layout = _prep_host(
        np.asarray(x), np.asarray(twiddle), np.asarray(bias)
    )
    if _emulate:
        outs = [_emulate_core(em) for em in emu_maps]
    else:
        from concourse.bass_utils import run_bass_kernel_spmd

        nc = _build_program(layout)
        res = run_bass_kernel_spmd(nc, in_maps, list(range(NCORES)), trace=_trace)
        _LAST_RESULTS["exec_time_ns"] = res.exec_time_ns
        _LAST_RESULTS["mean_exec_time_ns"] = res.mean_exec_time_ns
        outs = [r["o"] for r in res.results]
    full = np.concatenate(
        [
            np.asarray(o, dtype=np.float32)
            .reshape(OCL, P, NCH, B)
            .transpose(0, 3, 2, 1)
            .reshape(OCL, B, N)
            for o in outs
        ],
        axis=0,
    )
    return np.ascontiguousarray(full).reshape(B, OC, H, W).astype(np.float32)


def _build_program(layout):
    import concourse.bacc as bacc
    import concourse.mybir as mybir
    from concourse.tile import TileContext

    bf = mybir.dt.bfloat16
    f8 = mybir.dt.float8e4
    f32 = mybir.dt.float32
    MULT, ADD = mybir.AluOpType.mult, mybir.AluOpType.add
    G = B // DUP
    offs, lens, slot_pos = layout["offs"], layout["lens"], layout["slot_pos"]
    WTOT = layout["wtot"]
    WMAX = int(lens.max())

    nc = bacc.Bacc(None, target_bir_lowering=False)
    w_d = nc.dram_tensor("w", (P, WTOT), bf, kind="ExternalInput")
    ycf_d = nc.dram_tensor("ycf", (IC, P, YCF_LEN), bf, kind="ExternalInput")
    misc_d = nc.dram_tensor("misc", (P, 96), f32, kind="ExternalInput")
    o_d = nc.dram_tensor("o", (OCL, P, NCH * B), f32, kind="ExternalOutput")

    NB = NCH * B  # 512

    with TileContext(nc) as tc:
        with (
            tc.tile_pool(name="persist", bufs=1) as persist,
            tc.tile_pool(name="wpool", bufs=8) as wpool,
            tc.tile_pool(name="zcpool", bufs=3) as zcpool,
            tc.tile_pool(name="sb1", bufs=4) as sb1,
            tc.tile_pool(name="sb2", bufs=3) as sb2,
            tc.tile_pool(name="outp", bufs=1) as outp,
            tc.tile_pool(name="psz", bufs=4, space="PSUM") as psz,
            tc.tile_pool(name="psacc", bufs=OCL, space="PSUM") as psacc,
        ):
            misct = persist.tile([P, 96], f32, tag="misc")
            nc.scalar.dma_start(out=misct[:], in_=misc_d[:])
            biast = misct[:, 0:32]
            idt = misct[:, 32:96].bitcast(bf)  # [P, 128]

            accs = [psacc.tile([P, NB], f32, tag="acc", name=f"acc{i}") for i in range(OCL)]
            ycfts = [None] * IC

            # PE warm-up: dummy matmuls into accs[0] while the first weight
            # granule streams in, so the HAM clock-gate releases (1.2->2.4GHz)
            # before the real stage-A matmuls arrive.  The first identity
            # flush writes accs[0] with start=True, which resets PSUM and
            # discards this garbage.
            for _ in range(14):
                nc.tensor.matmul(
                    accs[0][:, 0:P],
                    idt,
                    idt,
                    start=True,
                    stop=True,
                    skip_group_check=True,
                )

            DELAY = 2
            deferred = []

            def flush_one():
                t9_, ic_, og_ = deferred.pop(0)
                for oo_ in range(2):
                    o_ = 2 * og_ + oo_
                    for e_ in range(2):
                        nc.tensor.matmul(
                            accs[o_][:],
                            idt,
                            t9_[:, (2 * oo_ + e_) * NB : (2 * oo_ + e_ + 1) * NB],
                            start=(ic_ == 0 and e_ == 0),
                            stop=(ic_ == IC - 1 and e_ == 1),
                        )

            ots = [None] * OCL

            def bias_add(og_):
                for oo in range(2):
                    o = 2 * og_ + oo
                    ots[o] = outp.tile([P, NB], f32, tag=f"out{o}", name=f"ot{o}")
                    nc.vector.tensor_tensor(
                        ots[o][:].rearrange("p (c b) -> p c b", c=NCH),
                        accs[o][:].rearrange("p (c b) -> p c b", c=NCH),
                        biast[:, o * NCH : (o + 1) * NCH]
                        .unsqueeze(2)
                        .broadcast_to((P, NCH, B)),
                        ADD,
                    )

            def store_out(og_):
                if og_ == 0:
                    # mid-run: scalar queue (sync still streams weights)
                    for oo in range(2):
                        o = 2 * og_ + oo
                        nc.scalar.dma_start(out=o_d[o], in_=ots[o][:])
                else:
                    # at the end: sync and gpsimd queues are idle; split each
                    # out in half across both so the tail transfer overlaps
                    h = NB // 2
                    for oo in range(2):
                        o = 2 * og_ + oo
                        nc.sync.dma_start(out=o_d[o][:, 0:h], in_=ots[o][:, 0:h])
                        nc.gpsimd.dma_start(
                            out=o_d[o][:, h:NB], in_=ots[o][:, h:NB]
                        )

            for og in range(NOG):
                for ic in range(IC):
                    if og == 0:
                        ycfts[ic] = persist.tile(
                            [P, YCF_LEN], bf, tag=f"ycf{ic}", name=f"ycf{ic}"
                        )
                        nc.gpsimd.dma_start(out=ycfts[ic][:], in_=ycf_d[ic])
                    ycft = ycfts[ic]
                    yt = ycft[:, YCF_Y:YCF_R8]

                    L = int(lens[og, ic])
                    base = int(offs[og, ic])
                    wt = wpool.tile([P, WMAX], bf, name="wt")
                    nc.sync.dma_start(out=wt[:, 0:L], in_=w_d[:, base : base + L])
                    wE8 = wt[:, 0:512].bitcast(f8)   # [P, 1024]

                    zc2 = zcpool.tile([P, 2 * NB], bf)
                    for oo in range(2):
                        z = psz.tile([P, NB], f32, tag="z", name=f"z{oo}")
                        for k in range(4):
                            nc.tensor.matmul(
                                z[:, (2 * k) * B : (2 * k + 2) * B],
                                wE8[:, (oo * 4 + k) * P : (oo * 4 + k + 1) * P],
                                yt[:, (2 * k) * B : (2 * k + 1) * B]
                                .unsqueeze(1)
                                .broadcast_to((P, 2, B)),
                                start=True,
                                stop=False,
                                skip_group_check=True,
                            )
                            for po in range(2):
                                isf8, pos = slot_pos[(og, ic, oo, k, po)]
                                if isf8:
                                    lhsT = wt[:, pos : pos + 64].bitcast(f8)
                                else:
                                    lhsT = wt[:, pos : pos + P]
                                nc.tensor.matmul(
                                    z[:, (2 * k + po) * B : (2 * k + po + 1) * B],
                                    lhsT,
                                    yt[:, (2 * k + 1) * B : (2 * k + 2) * B],
                                    start=False,
                                    stop=True,
                                    skip_group_check=True,
                                )
                        nc.scalar.activation(
                            zc2[:, oo * NB : (oo + 1) * NB],
                            z[:],
                            mybir.ActivationFunctionType.Copy,
                        )
                    # ---- L8/L9 elementwise ----
                    t8t = sb1.tile([P, 2 * NB], bf, tag="t8")
                    if GPS_T8:
                        # oc-half 0 on GpSimd with 4D APs (non-dup r8):
                        # out/in views: (k t n b); src chunk s1 = 4k+2+n
                        o0 = 2 * og
                        t8g = t8t[:, 0:NB].rearrange(
                            "p (k t n b) -> p k t n b", k=2, t=2, n=2
                        )
                        zcg = (
                            zc2[:, 0:NB]
                            .rearrange("p (k q n b) -> p k q n b", k=2, q=2, n=2)[
                                :, :, 1:2
                            ]
                            .broadcast_to((P, 2, 2, 2, B))
                        )
                        r8g = (
                            ycft[:, YCF_R8ND + o0 * NCH : YCF_R8ND + (o0 + 1) * NCH]
                            .rearrange("p (k t n) -> p k t n", k=2, t=2)
                            .unsqueeze(4)
                            .broadcast_to((P, 2, 2, 2, B))
                        )
                        nc.gpsimd.tensor_tensor(t8g, zcg, r8g, MULT)
                        # oc-half 1 on DVE (dup-2 packed)
                        sh8h = (P, 1, 2, 2, 2, G, DUP)
                        zq1 = zc2[:, NB : 2 * NB].rearrange(
                            "p (k q n g d) -> p k q n g d", k=2, q=2, n=2, d=DUP
                        )
                        r8o1 = (
                            ycft[
                                :,
                                YCF_R8 + (o0 + 1) * NCH * DUP : YCF_R8
                                + (o0 + 2) * NCH * DUP,
                            ]
                            .rearrange("p (k t n d) -> p k t n d", k=2, t=2, d=DUP)
                            .unsqueeze(4)
                            .broadcast_to((P, 2, 2, 2, G, DUP))
                        )
                        t8v1 = t8t[:, NB : 2 * NB].rearrange(
                            "p (k t n g d) -> p k t n g d", k=2, t=2, n=2, d=DUP
                        )
                        nc.vector.tensor_tensor(
                            t8v1,
                            zq1[:, :, 1:2, :, :, :].broadcast_to((P, 2, 2, 2, G, DUP)),
                            r8o1,
                            MULT,
                        )
                    else:
                        zq = zc2[:].rearrange(
                            "p (O k q n g d) -> p O k q n g d", O=2, k=2, q=2, n=2, d=DUP
                        )
                        sh8 = (P, 2, 2, 2, 2, G, DUP)
                        r8o = (
                            ycft[:, YCF_R8 + og * 2 * NCH * DUP : YCF_R8 + (og + 1) * 2 * NCH * DUP]
                            .rearrange("p (O k t n d) -> p O k t n d", O=2, k=2, t=2, d=DUP)
                            .unsqueeze(5)
                            .broadcast_to(sh8)
                        )
                        t8v = t8t[:].rearrange(
                            "p (O k t n g d) -> p O k t n g d", O=2, k=2, t=2, n=2, d=DUP
                        )
                        nc.vector.tensor_tensor(
                            t8v, zq[:, :, :, 1:2, :, :, :].broadcast_to(sh8), r8o, MULT
                        )
                    zq = zc2[:].rearrange(
                        "p (O k q n g d) -> p O k q n g d", O=2, k=2, q=2, n=2, d=DUP
                    )
                    sh8 = (P, 2, 2, 2, 2, G, DUP)
                    y8t = sb2.tile([P, 2 * NB], bf, tag="y8")
                    y8v = y8t[:].rearrange(
                        "p (O k t n g d) -> p O k t n g d", O=2, k=2, t=2, n=2, d=DUP
                    )
                    t8v = t8t[:].rearrange(
                        "p (O k t n g d) -> p O k t n g d", O=2, k=2, t=2, n=2, d=DUP
                    )
                    nc.vector.tensor_tensor(
                        y8v, zq[:, :, :, 0:1, :, :, :].broadcast_to(sh8), t8v, ADD
                    )
                    y8q = y8t[:].rearrange(
                        "p (O q m g d) -> p O q m g d", O=2, q=2, m=4, d=DUP
                    )
                    sh9 = (P, 2, 2, 2, 4, G, DUP)
                    v01o = (
                        ycft[:, YCF_V01 + og * 4 * NCH * DUP : YCF_V01 + (og + 1) * 4 * NCH * DUP]
                        .rearrange("p (O e t m d) -> p O e t m d", O=2, e=2, t=2, d=DUP)
                        .unsqueeze(5)
                        .broadcast_to(sh9)
                    )
                    t9 = sb1.tile([P, 4 * NB], bf, tag="t9")
                    t9v = t9[:].rearrange(
                        "p (O e t m g d) -> p O e t m g d", O=2, e=2, t=2, m=4, d=DUP
                    )
                    nc.vector.tensor_tensor(
                        t9v, y8q[:].unsqueeze(3).broadcast_to(sh9), v01o, MULT
                    )
                    deferred.append((t9, ic, og))
                    # taper the deferral near the end so the final flushes
                    # don't stack up serially after the last t9
                    dly = 1 if (og == 1 and ic >= IC - 3) else DELAY
                    while len(deferred) > dly:
                        flush_one()
                    # og0's bias happens mid-og1 so it doesn't stall the DVE
                    # stream at the og boundary; its out-DMA waits until the
                    # weight stream has drained (out transfers displace
                    # incoming w granules otherwise)
                    if og == 1 and ic == 3:
                        bias_add(0)
                    if og == 1 and ic == 12:
                        store_out(0)
            while deferred:
                flush_one()
            bias_add(1)
            store_out(1)
    nc.finalize()
    return nc


# revision 34
# speedup vs baseline: 1.0209x; 1.0209x over previous
"""Butterfly-Conv2d (nn_BConv2d) Trainium2 kernel, v6.

Math (reference): x(B=64,IC=16,32,32) -> y=x.reshape(IC,B,N=1024)[:,:,bitrev];
broadcast over OC=32; 10 radix-2 butterfly layers with per-(ic,oc) twiddles;
mean over ic; + bias -> (B,OC,32,32).

v6 (vs v5 ~121us):
  * DMA issue de-serialization: each dma_start occupies its issuing engine
    queue ~0.6us regardless of size (SWDGE descriptor generation), and v5
    serialized 118 of them on the Sync queue (~72us).  v6 merges everything
    into 32 weight-blob transfers (sync queue) + 16 y/coef transfers
    (gpsimd queue) + 1 misc + 4 outs (scalar queue).
  * Mixed-precision O slots: per-slot damage scores (quant-delta norm x
    downstream gain) are unioned across the 8 cores (the SPMD program is
    shared, so the fp8/bf16 position mask must be uniform; the row scales
    live in the data and differ per core).  The worst ~30% stay bf16, the
    rest go fp8 e4m3.  E slots are always fp8 (harmless).  Weight bytes:
    25.2MB (v4) -> 20.5MB (v5) -> ~14.5MB.
  * Per (og,ic) weights pack into one ragged bf16 blob: [E fp8 | O fp8 |
    O bf16], column offsets baked into the program; fp8 ranges read via
    AP.bitcast.  y/r8/v01/r8-nodup pack per ic into one [P,736] blob.
  * Optional GPSIMD offload (GPS_T8): one oc-half of the t8 mul per
    iteration runs on GpSimd (idle otherwise), shaving ~270ns/iter off the
    DVE critical chain.  Uses a non-dup r8 copy so every operand AP is <=4D.

Device layout: per ic tile [128, 512]: partition p = n & 127, free col =
C*64 + b with chunk C = n >> 7 (bits n9n8n7).
"""

import numpy as np
import ml_dtypes

B, IC, OC, H, W = 64, 16, 32, 32, 32
N = H * W          # 1024
NCORES = 8
OCL = OC // NCORES  # 4 oc per core
NOG = OCL // 2     # 2 og groups of 2 oc
NCH = 8            # chunks (n9n8n7)
P = 128            # partitions (n6..n0)
BF = ml_dtypes.bfloat16
F8 = ml_dtypes.float8_e4m3  # TRN FP8_EXP4: max finite 240
DUP = 2             # coefficient duplication for packed DVE reads
F8MAX = 240.0
FRAC_BF16 = 0.10    # fraction of O slots kept in bf16 (worst union damage)
                    # (emulated: 0.10 -> 6.3e-3, 0.15 -> 5.7e-3, gate 2e-2)
GPS_T8 = False      # offload one oc-half of the t8 mul to GpSimd
                    # (measured: GpSimd TTs contend for the shared SBUF port
                    # and slow DVE TTs ~24% -- net loss; keep False)

_EPS = np.float32(1e-20)

# ycf blob layout (bf16 units per partition)
YCF_Y, YCF_R8, YCF_V01, YCF_R8ND = 0, 512, 576, 704
YCF_LEN = 736


def _bitrev(n):
    bits = int(np.log2(n))
    idx = np.arange(n, dtype=np.int64)
    rev = np.zeros(n, dtype=np.int64)
    for b in range(bits):
        rev = (rev << 1) | ((idx >> b) & 1)
    return rev


def _compose_A7(tw):
    """Compose butterfly layers 0..6 into A[ic,oc,C,128,128] (C=8 chunks)."""
    ic, oc = tw.shape[0], tw.shape[1]
    A = np.zeros((ic, oc, NCH, P, P), dtype=np.float32)
    A[:] = np.eye(P, dtype=np.float32)
    for l in range(7):
        s = 1 << l
        nb_loc = P // (2 * s)
        t = tw[:, :, l].reshape(ic, oc, N // (2 * s), s, 2, 2)
        t = t.reshape(ic, oc, NCH, nb_loc, s, 2, 2)
        Av = A.reshape(ic, oc, NCH, nb_loc, 2, s, P)
        a0 = Av[:, :, :, :, 0]
        a1 = Av[:, :, :, :, 1]
        new0 = t[..., 0, 0, None] * a0 + t[..., 0, 1, None] * a1
        new1 = t[..., 1, 0, None] * a0 + t[..., 1, 1, None] * a1
        Av[:, :, :, :, 0] = new0
        Av[:, :, :, :, 1] = new1
    return A


def _safe(c):
    return np.where(np.abs(c) < _EPS, np.where(c < 0, -_EPS, _EPS), c)


def _stageB_coeffs(tw):
    """Ratio-trick coefficients for layers 7,8,9: each [ic,oc,P,NCH] f32."""
    ic, oc = tw.shape[0], tw.shape[1]
    pr = np.arange(P)
    cp = np.arange(NCH)

    t7 = tw[:, :, 7].reshape(ic, oc, 4, P, 2, 2)      # [k, p, po, q]
    k7, po7 = cp >> 1, cp & 1
    c0 = t7[:, :, k7[None, :], pr[:, None], po7[None, :], 0]
    c1 = t7[:, :, k7[None, :], pr[:, None], po7[None, :], 1]
    r7 = (c1 / _safe(c0)).astype(np.float32)
    pend7 = c0

    t8 = tw[:, :, 8].reshape(ic, oc, 2, 256, 2, 2)    # [k9, n7*128+p, po, q]
    k8, po8, n78 = cp >> 2, (cp >> 1) & 1, cp & 1
    s8 = np.stack([4 * k8 + 0 + n78, 4 * k8 + 2 + n78])
    u0 = t8[:, :, k8[None, :], n78[None, :] * P + pr[:, None], po8[None, :], 0]
    u1 = t8[:, :, k8[None, :], n78[None, :] * P + pr[:, None], po8[None, :], 1]
    e0 = u0 * pend7[:, :, :, s8[0]]
    e1 = u1 * pend7[:, :, :, s8[1]]
    r8 = (e1 / _safe(e0)).astype(np.float32)
    pend8 = e0

    t9 = tw[:, :, 9].reshape(ic, oc, 512, 2, 2)       # [m*128+p, po, q]
    po9, m9 = cp >> 2, cp & 3
    s9 = np.stack([m9, 4 + m9])
    w0 = t9[:, :, m9[None, :] * P + pr[:, None], po9[None, :], 0]
    w1 = t9[:, :, m9[None, :] * P + pr[:, None], po9[None, :], 1]
    v0 = (w0 * pend8[:, :, :, s9[0]] / IC).astype(np.float32)
    v1 = (w1 * pend8[:, :, :, s9[1]] / IC).astype(np.float32)

    return r7, r8, v0, v1


# static source maps (twiddle independent)
_CP = np.arange(NCH)
S8 = np.stack([4 * (_CP >> 2) + (_CP & 1), 4 * (_CP >> 2) + 2 + (_CP & 1)])
S9 = np.stack([_CP & 3, 4 + (_CP & 3)])
_G0 = (S8[0] >> 1).astype(np.int64)
_G1 = (S8[1] >> 1).astype(np.int64)


def _f8(x):
    return np.clip(
        np.clip(x, -F8MAX, F8MAX).astype(F8).astype(np.float32), -F8MAX, F8MAX
    )


def _slot_scores(lhsT_E, O, r8, v0, v1):
    """Union-over-cores damage score of e4m3-quantizing each O slot.

    score[ic,oc,k,po] ~ max_m ||quant_delta[:,m]||_2 * gain_zc[m, 2k+po]
    where gain_zc bounds |d out / d zc| through the L8/L9 coefficient paths.
    Returns mask[NOG, IC, 2, 4, 2] (True = keep bf16), uniform across cores.
    """
    Emax = np.abs(lhsT_E).max(axis=3)                  # [ic,oc,k,m]
    Omax = np.abs(O).max(axis=4)                       # [ic,oc,k,po,m]
    gain_y8p = np.zeros((IC, OC, P, NCH), np.float32)
    vs = (v0, v1)
    for e in range(2):
        for C in range(NCH):
            gain_y8p[:, :, :, S9[e][C]] += np.abs(vs[e][:, :, :, C])
    gain_zc = np.zeros((IC, OC, P, NCH), np.float32)
    for Cpp in range(NCH):
        gain_zc[:, :, :, S8[0][Cpp]] += gain_y8p[:, :, :, Cpp]
        gain_zc[:, :, :, S8[1][Cpp]] += gain_y8p[:, :, :, Cpp] * np.abs(
            r8[:, :, :, Cpp]
        )
    score = np.zeros((IC, OC, 4, 2), np.float32)
    for k in range(4):
        for po in range(2):
            Ss = F8MAX / np.maximum(
                np.maximum(Emax[:, :, k], Omax[:, :, k, po]), _EPS
            )
            v = O[:, :, k, po] * Ss[:, :, None, :]
            delta = (_f8(v) - v) / Ss[:, :, None, :]
            dn = np.linalg.norm(delta, axis=2)          # [ic,oc,m]
            score[:, :, k, po] = (dn * gain_zc[:, :, :, 2 * k + po]).max(axis=2)
    # positions: oc = core*OCL + 2*og + oo
    sc = score.reshape(IC, NCORES, NOG, 2, 4, 2).transpose(1, 2, 0, 3, 4, 5)
    vote = sc.max(axis=0)                               # [og, ic, oo, k, po]
    thr = np.quantile(vote, 1.0 - FRAC_BF16)
    mask = vote >= thr
    # cap bf16 slots per (og,ic) granule at 5 so granule sizes (and hence
    # per-iteration DMA arrival times) stay roughly uniform
    for og in range(NOG):
        for ic in range(IC):
            g = mask[og, ic]
            n = int(g.sum())
            if n > 5:
                v = vote[og, ic].ravel()
                keep = np.argsort(v)[::-1][:5]
                g2 = np.zeros(16, bool)
                g2[keep] = True
                mask[og, ic] = g2.reshape(2, 4, 2)
    # force the first granule all-fp8 so the first weight transfer is small
    # and the pipeline starts earlier
    mask[0, 0] = False
    return mask


def _prep_host(x, twiddle, bias):
    perm = _bitrev(N)
    y = np.ascontiguousarray(x).reshape(IC, B, N)[:, :, perm]
    y_dev = np.ascontiguousarray(
        y.reshape(IC, B, NCH, P).transpose(0, 3, 2, 1)
    ).reshape(IC, P, NCH * B).astype(BF)

    tw = np.asarray(twiddle, dtype=np.float32)
    A = _compose_A7(tw)                    # [IC, OC, NCH, P(out), P(in)]
    r7, r8, v0, v1 = _stageB_coeffs(tw)    # each [IC, OC, P, NCH]

    lhsT_E = A[:, :, 0::2].transpose(0, 1, 2, 4, 3)          # [ic,oc,4,p_in,m]
    lhsT_O = A[:, :, 1::2].transpose(0, 1, 2, 4, 3)
    r7m = r7.transpose(0, 1, 3, 2)                            # [ic,oc,C',m]
    O = lhsT_O[:, :, :, None] * r7m.reshape(IC, OC, 4, 2, 1, P)  # [ic,oc,k,po,p,m]

    mask16 = _slot_scores(lhsT_E, O, r8, v0, v1)   # [og,ic,oo,k,po] True=bf16

    # ragged blob layout, shared across cores: per (og,ic):
    #   [E: 512 u] [fp8 O slots: 64 u each] [bf16 O slots: 128 u each]
    # (u = bf16 unit = 2 bytes)
    offs = np.zeros((NOG, IC), np.int64)
    lens = np.zeros((NOG, IC), np.int64)
    cur = 0
    slot_pos = {}
    for og in range(NOG):
        for ic in range(IC):
            offs[og, ic] = cur
            n8 = int((~mask16[og, ic]).sum())
            i8 = i16 = 0
            for oo in range(2):
                for k in range(4):
                    for po in range(2):
                        if mask16[og, ic, oo, k, po]:
                            slot_pos[(og, ic, oo, k, po)] = (False, 512 + n8 * 64 + i16 * 128)
                            i16 += 1
                        else:
                            slot_pos[(og, ic, oo, k, po)] = (True, 512 + i8 * 64)
                            i8 += 1
            lens[og, ic] = 512 + n8 * 64 + i16 * 128
            cur += lens[og, ic]
    WTOT = int(cur)
    layout = {"offs": offs, "lens": lens, "slot_pos": slot_pos, "wtot": WTOT}

    bias_np = np.asarray(bias, dtype=np.float32).reshape(OC, NCH, P)

    in_maps, emu_maps = [], []
    for core in range(NCORES):
        osl = slice(core * OCL, (core + 1) * OCL)
        Eg = lhsT_E[:, osl]                     # [ic,ocl,k,p,m]
        Og = O[:, osl]                          # [ic,ocl,k,po,p,m]
        Emax = np.abs(Eg).max(axis=3)           # [ic,ocl,k,m]
        Omax = np.abs(Og).max(axis=4)           # [ic,ocl,k,po,m]
        # group scale covers E plus any fp8 O slot of the group
        m_oc = np.zeros((IC, OCL, 4, 2), bool)  # True = bf16
        for og in range(NOG):
            for oo in range(2):
                m_oc[:, 2 * og + oo] = mask16[og, :, oo]
        incl = np.where(m_oc.transpose(0, 1, 2, 3)[:, :, :, :, None], 0.0, Omax)
        gm = np.maximum(Emax, incl.max(axis=3))
        S = (F8MAX / np.maximum(gm, _EPS)).astype(np.float32)  # [ic,ocl,k,m]

        wEq = _f8(Eg * S[:, :, :, None, :])
        wOs = Og * S[:, :, :, None, None, :]
        for k in range(4):
            for po in range(2):
                sel = ~m_oc[:, :, k, po]
                wq = _f8(wOs[:, :, k, po])
                wOs[:, :, k, po] = np.where(sel[:, :, None, None], wq, wOs[:, :, k, po])

        Sp = S.transpose(0, 1, 3, 2)            # [ic,ocl,p,k]
        r8f = r8[:, osl] * (Sp[:, :, :, _G0] / Sp[:, :, :, _G1])
        v0f = v0[:, osl] / Sp[:, :, :, _G0[S9[0]]]
        v1f = v1[:, osl] / Sp[:, :, :, _G0[S9[1]]]

        # pack w blob
        wb = np.zeros((P, WTOT), dtype=BF)
        wb_u8 = wb.view(np.uint8).reshape(P, WTOT * 2)
        for og in range(NOG):
            for ic in range(IC):
                base = int(offs[og, ic])
                # E: [oo,k] fp8, 128 cols each -> 512 units
                eb = np.ascontiguousarray(
                    wEq[ic, 2 * og : 2 * og + 2].transpose(2, 0, 1, 3)
                ).reshape(P, 1024).astype(F8)
                wb_u8[:, base * 2 : base * 2 + 1024] = eb.view(np.uint8)
                for oo in range(2):
                    for k in range(4):
                        for po in range(2):
                            isf8, pos = slot_pos[(og, ic, oo, k, po)]
                            sl = np.ascontiguousarray(wOs[ic, 2 * og + oo, k, po])
                            bo = (base + pos) * 2
                            if isf8:
                                wb_u8[:, bo : bo + 128] = sl.astype(F8).view(np.uint8)
                            else:
                                wb_u8[:, bo : bo + 256] = sl.astype(BF).view(np.uint8)

        # ycf blob: y 512 | r8 dup 64 | v01 dup 128 | r8 nodup 32 (pad 736)
        ycf = np.zeros((IC, P, YCF_LEN), dtype=BF)
        ycf[:, :, YCF_Y:YCF_R8] = y_dev
        r8d = np.repeat(
            r8f.transpose(0, 2, 1, 3)[..., None], DUP, axis=-1
        ).reshape(IC, P, OCL * NCH * DUP)
        ycf[:, :, YCF_R8:YCF_V01] = r8d.astype(BF)
        v01d = np.repeat(
            np.stack([v0f, v1f], axis=2).transpose(0, 3, 1, 2, 4)[..., None],
            DUP, axis=-1,
        ).reshape(IC, P, OCL * 2 * NCH * DUP)
        ycf[:, :, YCF_V01:YCF_R8ND] = v01d.astype(BF)
        r8nd = r8f.transpose(0, 2, 1, 3).reshape(IC, P, OCL * NCH)
        ycf[:, :, YCF_R8ND : YCF_R8ND + OCL * NCH] = r8nd.astype(BF)

        # misc blob f32 [P, 96]: bias 32 f32 | ident 128 bf16 (64 f32 units)
        misc = np.zeros((P, 96), dtype=np.float32)
        misc[:, :32] = np.ascontiguousarray(
            bias_np[osl].transpose(2, 0, 1)
        ).reshape(P, OCL * NCH)
        ident = np.eye(P, dtype=np.float32).astype(BF)
        misc.view(np.uint8)[:, 128:384] = ident.view(np.uint8)

        in_maps.append({"w": wb, "ycf": ycf, "misc": misc})
        emu_maps.append(
            {
                "y": y_dev,
                "wE": wEq,      # f32 values (exactly fp8)
                "wO": wOs,      # f32 values (fp8 or bf16-roundable)
                "r8": r8f,
                "v01": (v0f, v1f),
                "bias": misc[:, :32],
            }
        )
    return in_maps, emu_maps, layout


def _emulate_core(em):
    """Numpy emulation of the device program (exact op/rounding semantics)."""
    y = em["y"].reshape(IC, P, NCH, B)
    wE, wO = em["wE"], em["wO"]
    r8a = em["r8"]
    v0a, v1a = em["v01"]
    bias = em["bias"].reshape(P, OCL, NCH)
    out = np.zeros((OCL, P, NCH, B), dtype=np.float32)
    for o in range(OCL):
        acc = np.zeros((P, NCH, B), dtype=np.float32)
        for ic in range(IC):
            yv = y[ic].astype(np.float32)
            z = np.zeros((P, NCH, B), dtype=np.float32)
            for Cp in range(NCH):
                k, po = Cp >> 1, Cp & 1
                lE = wE[ic, o, k]
                lO = wO[ic, o, k, po].astype(BF).astype(np.float32)
                z[:, Cp] = lE.T @ yv[:, 2 * k] + lO.T @ yv[:, 2 * k + 1]
            zc = z.astype(BF).astype(np.float32)
            r8 = r8a[ic, o].astype(BF).astype(np.float32)
            v0 = v0a[ic, o].astype(BF).astype(np.float32)
            v1 = v1a[ic, o].astype(BF).astype(np.float32)
            tmp8 = (r8[:, :, None] * zc[:, S8[1]]).astype(BF).astype(np.float32)
            y8p = (zc[:, S8[0]] + tmp8).astype(BF).astype(np.float32)
            t9a = (v0[:, :, None] * y8p[:, S9[0]]).astype(BF).astype(np.float32)
            t9b = (v1[:, :, None] * y8p[:, S9[1]]).astype(BF).astype(np.float32)
            acc += t9a + t9b
        out[o] = acc + bias[:, o, :, None]
    return out.reshape(OCL, P, NCH * B)


_LAST_RESULTS = {"exec_time_ns": None}


def kernel(x, twiddle, bias, _trace=False, _emulate=False):
    in_maps, emu_maps, layout = _prep_host(
        np.asarray(x), np.asarray(twiddle), np.asarray(bias)
    )
    if _emulate:
        outs = [_emulate_core(em) for em in emu_maps]
    else:
        from concourse.bass_utils import run_bass_kernel_spmd

        nc = _build_program(layout)
        res = run_bass_kernel_spmd(nc, in_maps, list(range(NCORES)), trace=_trace)
        _LAST_RESULTS["exec_time_ns"] = res.exec_time_ns
        _LAST_RESULTS["mean_exec_time_ns"] = res.mean_exec_time_ns
        outs = [r["o"] for r in res.results]
    full = np.concatenate(
        [
            np.asarray(o, dtype=np.float32)
            .reshape(OCL, P, NCH, B)
            .transpose(0, 3, 2, 1)
            .reshape(OCL, B, N)
            for o in outs
        ],
        axis=0,
    )
    return np.ascontiguousarray(full).reshape(B, OC, H, W).astype(np.float32)


def _build_program(layout):
    import concourse.bacc as bacc
    import concourse.mybir as mybir
    from concourse.tile import TileContext

    bf = mybir.dt.bfloat16
    f8 = mybir.dt.float8e4
    f32 = mybir.dt.float32
    MULT, ADD = mybir.AluOpType.mult, mybir.AluOpType.add
    G = B // DUP
    offs, lens, slot_pos = layout["offs"], layout["lens"], layout["slot_pos"]
    WTOT = layout["wtot"]
    WMAX = int(lens.max())

    nc = bacc.Bacc(None, target_bir_lowering=False)
    w_d = nc.dram_tensor("w", (P, WTOT), bf, kind="ExternalInput")
    ycf_d = nc.dram_tensor("ycf", (IC, P, YCF_LEN), bf, kind="ExternalInput")
    misc_d = nc.dram_tensor("misc", (P, 96), f32, kind="ExternalInput")
    o_d = nc.dram_tensor("o", (OCL, P, NCH * B), f32, kind="ExternalOutput")

    NB = NCH * B  # 512

    with TileContext(nc) as tc:
        with (
            tc.tile_pool(name="persist", bufs=1) as persist,
            tc.tile_pool(name="wpool", bufs=8) as wpool,
            tc.tile_pool(name="zcpool", bufs=3) as zcpool,
            tc.tile_pool(name="sb1", bufs=4) as sb1,
            tc.tile_pool(name="sb2", bufs=3) as sb2,
            tc.tile_pool(name="outp", bufs=1) as outp,
            tc.tile_pool(name="psz", bufs=4, space="PSUM") as psz,
            tc.tile_pool(name="psacc", bufs=OCL, space="PSUM") as psacc,
        ):
            misct = persist.tile([P, 96], f32, tag="misc")
            nc.scalar.dma_start(out=misct[:], in_=misc_d[:])
            biast = misct[:, 0:32]
            idt = misct[:, 32:96].bitcast(bf)  # [P, 128]

            accs = [psacc.tile([P, NB], f32, tag="acc", name=f"acc{i}") for i in range(OCL)]
            ycfts = [None] * IC

            # PE warm-up: dummy matmuls into accs[0] while the first weight
            # granule streams in, so the HAM clock-gate releases (1.2->2.4GHz)
            # before the real stage-A matmuls arrive.  The first identity
            # flush writes accs[0] with start=True, which resets PSUM and
            # discards this garbage.
            for _ in range(14):
                nc.tensor.matmul(
                    accs[0][:, 0:P],
                    idt,
                    idt,
                    start=True,
                    stop=True,
                    skip_group_check=True,
                )

            DELAY = 2
            deferred = []

            def flush_one():
                t9_, ic_, og_ = deferred.pop(0)
                for oo_ in range(2):
                    o_ = 2 * og_ + oo_
                    for e_ in range(2):
                        nc.tensor.matmul(
                            accs[o_][:],
                            idt,
                            t9_[:, (2 * oo_ + e_) * NB : (2 * oo_ + e_ + 1) * NB],
                            start=(ic_ == 0 and e_ == 0),
                            stop=(ic_ == IC - 1 and e_ == 1),
                        )

            ots = [None] * OCL

            def bias_add(og_):
                for oo in range(2):
                    o = 2 * og_ + oo
                    ots[o] = outp.tile([P, NB], f32, tag=f"out{o}", name=f"ot{o}")
                    nc.vector.tensor_tensor(
                        ots[o][:].rearrange("p (c b) -> p c b", c=NCH),
                        accs[o][:].rearrange("p (c b) -> p c b", c=NCH),
                        biast[:, o * NCH : (o + 1) * NCH]
                        .unsqueeze(2)
                        .broadcast_to((P, NCH, B)),
                        ADD,
                    )

            def store_out(og_):
                # og0 mid-run: scalar queue (sync still streams weights);
                # og1 at the end: sync queue is idle by then
                eng = nc.scalar if og_ == 0 else nc.sync
                for oo in range(2):
                    o = 2 * og_ + oo
                    eng.dma_start(out=o_d[o], in_=ots[o][:])

            for og in range(NOG):
                for ic in range(IC):
                    if og == 0:
                        ycfts[ic] = persist.tile(
                            [P, YCF_LEN], bf, tag=f"ycf{ic}", name=f"ycf{ic}"
                        )
                        nc.gpsimd.dma_start(out=ycfts[ic][:], in_=ycf_d[ic])
                    ycft = ycfts[ic]
                    yt = ycft[:, YCF_Y:YCF_R8]

                    L = int(lens[og, ic])
                    base = int(offs[og, ic])
                    wt = wpool.tile([P, WMAX], bf, name="wt")
                    nc.sync.dma_start(out=wt[:, 0:L], in_=w_d[:, base : base + L])
                    wE8 = wt[:, 0:512].bitcast(f8)   # [P, 1024]

                    zc2 = zcpool.tile([P, 2 * NB], bf)
                    for oo in range(2):
                        z = psz.tile([P, NB], f32, tag="z", name=f"z{oo}")
                        for k in range(4):
                            nc.tensor.matmul(
                                z[:, (2 * k) * B : (2 * k + 2) * B],
                                wE8[:, (oo * 4 + k) * P : (oo * 4 + k + 1) * P],
                                yt[:, (2 * k) * B : (2 * k + 1) * B]
                                .unsqueeze(1)
                                .broadcast_to((P, 2, B)),
                                start=True,
                                stop=False,
                                skip_group_check=True,
                            )
                            for po in range(2):
                                isf8, pos = slot_pos[(og, ic, oo, k, po)]
                                if isf8:
                                    lhsT = wt[:, pos : pos + 64].bitcast(f8)
                                else:
                                    lhsT = wt[:, pos : pos + P]
                                nc.tensor.matmul(
                                    z[:, (2 * k + po) * B : (2 * k + po + 1) * B],
                                    lhsT,
                                    yt[:, (2 * k + 1) * B : (2 * k + 2) * B],
                                    start=False,
                                    stop=True,
                                    skip_group_check=True,
                                )
                        nc.scalar.activation(
                            zc2[:, oo * NB : (oo + 1) * NB],
                            z[:],
                            mybir.ActivationFunctionType.Copy,
                        )
                    # ---- L8/L9 elementwise ----
                    t8t = sb1.tile([P, 2 * NB], bf, tag="t8")
                    if GPS_T8:
                        # oc-half 0 on GpSimd with 4D APs (non-dup r8):
                        # out/in views: (k t n b); src chunk s1 = 4k+2+n
                        o0 = 2 * og
                        t8g = t8t[:, 0:NB].rearrange(
                            "p (k t n b) -> p k t n b", k=2, t=2, n=2
                        )
                        zcg = (
                            zc2[:, 0:NB]
                            .rearrange("p (k q n b) -> p k q n b", k=2, q=2, n=2)[
                                :, :, 1:2
                            ]
                            .broadcast_to((P, 2, 2, 2, B))
                        )
                        r8g = (
                            ycft[:, YCF_R8ND + o0 * NCH : YCF_R8ND + (o0 + 1) * NCH]
                            .rearrange("p (k t n) -> p k t n", k=2, t=2)
                            .unsqueeze(4)
                            .broadcast_to((P, 2, 2, 2, B))
                        )
                        nc.gpsimd.tensor_tensor(t8g, zcg, r8g, MULT)
                        # oc-half 1 on DVE (dup-2 packed)
                        sh8h = (P, 1, 2, 2, 2, G, DUP)
                        zq1 = zc2[:, NB : 2 * NB].rearrange(
                            "p (k q n g d) -> p k q n g d", k=2, q=2, n=2, d=DUP
                        )
                        r8o1 = (
                            ycft[
                                :,
                                YCF_R8 + (o0 + 1) * NCH * DUP : YCF_R8
                                + (o0 + 2) * NCH * DUP,
                            ]
                            .rearrange("p (k t n d) -> p k t n d", k=2, t=2, d=DUP)
                            .unsqueeze(4)
                            .broadcast_to((P, 2, 2, 2, G, DUP))
                        )
                        t8v1 = t8t[:, NB : 2 * NB].rearrange(
                            "p (k t n g d) -> p k t n g d", k=2, t=2, n=2, d=DUP
                        )
                        nc.vector.tensor_tensor(
                            t8v1,
                            zq1[:, :, 1:2, :, :, :].broadcast_to((P, 2, 2, 2, G, DUP)),
                            r8o1,
                            MULT,
                        )
                    else:
                        zq = zc2[:].rearrange(
                            "p (O k q n g d) -> p O k q n g d", O=2, k=2, q=2, n=2, d=DUP
                        )
                        sh8 = (P, 2, 2, 2, 2, G, DUP)
                        r8o = (
                            ycft[:, YCF_R8 + og * 2 * NCH * DUP : YCF_R8 + (og + 1) * 2 * NCH * DUP]
                            .rearrange("p (O k t n d) -> p O k t n d", O=2, k=2, t=2, d=DUP)
                            .unsqueeze(5)
                            .broadcast_to(sh8)
                        )
                        t8v = t8t[:].rearrange(
                            "p (O k t n g d) -> p O k t n g d", O=2, k=2, t=2, n=2, d=DUP
                        )
                        nc.vector.tensor_tensor(
                            t8v, zq[:, :, :, 1:2, :, :, :].broadcast_to(sh8), r8o, MULT
                        )
                    zq = zc2[:].rearrange(
                        "p (O k q n g d) -> p O k q n g d", O=2, k=2, q=2, n=2, d=DUP
                    )
                    sh8 = (P, 2, 2, 2, 2, G, DUP)
                    y8t = sb2.tile([P, 2 * NB], bf, tag="y8")
                    y8v = y8t[:].rearrange(
                        "p (O k t n g d) -> p O k t n g d", O=2, k=2, t=2, n=2, d=DUP
                    )
                    t8v = t8t[:].rearrange(
                        "p (O k t n g d) -> p O k t n g d", O=2, k=2, t=2, n=2, d=DUP
                    )
                    nc.vector.tensor_tensor(
                        y8v, zq[:, :, :, 0:1, :, :, :].broadcast_to(sh8), t8v, ADD
                    )
                    y8q = y8t[:].rearrange(
                        "p (O q m g d) -> p O q m g d", O=2, q=2, m=4, d=DUP
                    )
                    sh9 = (P, 2, 2, 2, 4, G, DUP)
                    v01o = (
                        ycft[:, YCF_V01 + og * 4 * NCH * DUP : YCF_V01 + (og + 1) * 4 * NCH * DUP]
                        .rearrange("p (O e t m d) -> p O e t m d", O=2, e=2, t=2, d=DUP)
                        .unsqueeze(5)
                        .broadcast_to(sh9)
                    )
                    t9 = sb1.tile([P, 4 * NB], bf, tag="t9")
                    t9v = t9[:].rearrange(
                        "p (O e t m g d) -> p O e t m g d", O=2, e=2, t=2, m=4, d=DUP
                    )
                    nc.vector.tensor_tensor(
                        t9v, y8q[:].unsqueeze(3).broadcast_to(sh9), v01o, MULT
                    )
                    deferred.append((t9, ic, og))
                    # taper the deferral near the end so the final flushes
                    # don't stack up serially after the last t9
                    dly = 1 if (og == 1 and ic >= IC - 3) else DELAY
                    while len(deferred) > dly:
                        flush_one()
                    # og0's bias happens mid-og1 so it doesn't stall the DVE
                    # stream at the og boundary; its out-DMA waits until the
                    # weight stream has drained (out transfers displace
                    # incoming w granules otherwise)
                    if og == 1 and ic == 3:
                        bias_add(0)
                    if og == 1 and ic == 12:
                        store_out(0)
            while deferred:
                flush_one()
            bias_add(1)
            store_out(1)
    nc.finalize()
    return nc


# revision 36
# speedup vs baseline: 1.0224x; 1.0014x over previous
"""Butterfly-Conv2d (nn_BConv2d) Trainium2 kernel, v6.

Math (reference): x(B=64,IC=16,32,32) -> y=x.reshape(IC,B,N=1024)[:,:,bitrev];
broadcast over OC=32; 10 radix-2 butterfly layers with per-(ic,oc) twiddles;
mean over ic; + bias -> (B,OC,32,32).

v6 (vs v5 ~121us):
  * DMA issue de-serialization: each dma_start occupies its issuing engine
    queue ~0.6us regardless of size (SWDGE descriptor generation), and v5
    serialized 118 of them on the Sync queue (~72us).  v6 merges everything
    into 32 weight-blob transfers (sync queue) + 16 y/coef transfers
    (gpsimd queue) + 1 misc + 4 outs (scalar queue).
  * Mixed-precision O slots: per-slot damage scores (quant-delta norm x
    downstream gain) are unioned across the 8 cores (the SPMD program is
    shared, so the fp8/bf16 position mask must be uniform; the row scales
    live in the data and differ per core).  The worst ~30% stay bf16, the
    rest go fp8 e4m3.  E slots are always fp8 (harmless).  Weight bytes:
    25.2MB (v4) -> 20.5MB (v5) -> ~14.5MB.
  * Per (og,ic) weights pack into one ragged bf16 blob: [E fp8 | O fp8 |
    O bf16], column offsets baked into the program; fp8 ranges read via
    AP.bitcast.  y/r8/v01/r8-nodup pack per ic into one [P,736] blob.
  * Optional GPSIMD offload (GPS_T8): one oc-half of the t8 mul per
    iteration runs on GpSimd (idle otherwise), shaving ~270ns/iter off the
    DVE critical chain.  Uses a non-dup r8 copy so every operand AP is <=4D.

Device layout: per ic tile [128, 512]: partition p = n & 127, free col =
C*64 + b with chunk C = n >> 7 (bits n9n8n7).
"""

import numpy as np
import ml_dtypes

B, IC, OC, H, W = 64, 16, 32, 32, 32
N = H * W          # 1024
NCORES = 8
OCL = OC // NCORES  # 4 oc per core
NOG = OCL // 2     # 2 og groups of 2 oc
NCH = 8            # chunks (n9n8n7)
P = 128            # partitions (n6..n0)
BF = ml_dtypes.bfloat16
F8 = ml_dtypes.float8_e4m3  # TRN FP8_EXP4: max finite 240
DUP = 2             # coefficient duplication for packed DVE reads
F8MAX = 240.0
FRAC_BF16 = 0.10    # fraction of O slots kept in bf16 (worst union damage)
                    # (emulated: 0.10 -> 6.3e-3, 0.15 -> 5.7e-3, gate 2e-2)
GPS_T8 = False      # offload one oc-half of the t8 mul to GpSimd
                    # (measured: GpSimd TTs contend for the shared SBUF port
                    # and slow DVE TTs ~24% -- net loss; keep False)

_EPS = np.float32(1e-20)

# ycf blob layout (bf16 units per partition)
YCF_Y, YCF_R8, YCF_V01, YCF_R8ND = 0, 512, 576, 704
YCF_LEN = 736


def _bitrev(n):
    bits = int(np.log2(n))
    idx = np.arange(n, dtype=np.int64)
    rev = np.zeros(n, dtype=np.int64)
    for b in range(bits):
        rev = (rev << 1) | ((idx >> b) & 1)
    return rev


def _compose_A7(tw):
    """Compose butterfly layers 0..6 into A[ic,oc,C,128,128] (C=8 chunks)."""
    ic, oc = tw.shape[0], tw.shape[1]
    A = np.zeros((ic, oc, NCH, P, P), dtype=np.float32)
    A[:] = np.eye(P, dtype=np.float32)
    for l in range(7):
        s = 1 << l
        nb_loc = P // (2 * s)
        t = tw[:, :, l].reshape(ic, oc, N // (2 * s), s, 2, 2)
        t = t.reshape(ic, oc, NCH, nb_loc, s, 2, 2)
        Av = A.reshape(ic, oc, NCH, nb_loc, 2, s, P)
        a0 = Av[:, :, :, :, 0]
        a1 = Av[:, :, :, :, 1]
        new0 = t[..., 0, 0, None] * a0 + t[..., 0, 1, None] * a1
        new1 = t[..., 1, 0, None] * a0 + t[..., 1, 1, None] * a1
        Av[:, :, :, :, 0] = new0
        Av[:, :, :, :, 1] = new1
    return A


def _safe(c):
    return np.where(np.abs(c) < _EPS, np.where(c < 0, -_EPS, _EPS), c)


def _stageB_coeffs(tw):
    """Ratio-trick coefficients for layers 7,8,9: each [ic,oc,P,NCH] f32."""
    ic, oc = tw.shape[0], tw.shape[1]
    pr = np.arange(P)
    cp = np.arange(NCH)

    t7 = tw[:, :, 7].reshape(ic, oc, 4, P, 2, 2)      # [k, p, po, q]
    k7, po7 = cp >> 1, cp & 1
    c0 = t7[:, :, k7[None, :], pr[:, None], po7[None, :], 0]
    c1 = t7[:, :, k7[None, :], pr[:, None], po7[None, :], 1]
    r7 = (c1 / _safe(c0)).astype(np.float32)
    pend7 = c0

    t8 = tw[:, :, 8].reshape(ic, oc, 2, 256, 2, 2)    # [k9, n7*128+p, po, q]
    k8, po8, n78 = cp >> 2, (cp >> 1) & 1, cp & 1
    s8 = np.stack([4 * k8 + 0 + n78, 4 * k8 + 2 + n78])
    u0 = t8[:, :, k8[None, :], n78[None, :] * P + pr[:, None], po8[None, :], 0]
    u1 = t8[:, :, k8[None, :], n78[None, :] * P + pr[:, None], po8[None, :], 1]
    e0 = u0 * pend7[:, :, :, s8[0]]
    e1 = u1 * pend7[:, :, :, s8[1]]
    r8 = (e1 / _safe(e0)).astype(np.float32)
    pend8 = e0

    t9 = tw[:, :, 9].reshape(ic, oc, 512, 2, 2)       # [m*128+p, po, q]
    po9, m9 = cp >> 2, cp & 3
    s9 = np.stack([m9, 4 + m9])
    w0 = t9[:, :, m9[None, :] * P + pr[:, None], po9[None, :], 0]
    w1 = t9[:, :, m9[None, :] * P + pr[:, None], po9[None, :], 1]
    v0 = (w0 * pend8[:, :, :, s9[0]] / IC).astype(np.float32)
    v1 = (w1 * pend8[:, :, :, s9[1]] / IC).astype(np.float32)

    return r7, r8, v0, v1


# static source maps (twiddle independent)
_CP = np.arange(NCH)
S8 = np.stack([4 * (_CP >> 2) + (_CP & 1), 4 * (_CP >> 2) + 2 + (_CP & 1)])
S9 = np.stack([_CP & 3, 4 + (_CP & 3)])
_G0 = (S8[0] >> 1).astype(np.int64)
_G1 = (S8[1] >> 1).astype(np.int64)


def _f8(x):
    return np.clip(
        np.clip(x, -F8MAX, F8MAX).astype(F8).astype(np.float32), -F8MAX, F8MAX
    )


def _slot_scores(lhsT_E, O, r8, v0, v1):
    """Union-over-cores damage score of e4m3-quantizing each O slot.

    score[ic,oc,k,po] ~ max_m ||quant_delta[:,m]||_2 * gain_zc[m, 2k+po]
    where gain_zc bounds |d out / d zc| through the L8/L9 coefficient paths.
    Returns mask[NOG, IC, 2, 4, 2] (True = keep bf16), uniform across cores.
    """
    Emax = np.abs(lhsT_E).max(axis=3)                  # [ic,oc,k,m]
    Omax = np.abs(O).max(axis=4)                       # [ic,oc,k,po,m]
    gain_y8p = np.zeros((IC, OC, P, NCH), np.float32)
    vs = (v0, v1)
    for e in range(2):
        for C in range(NCH):
            gain_y8p[:, :, :, S9[e][C]] += np.abs(vs[e][:, :, :, C])
    gain_zc = np.zeros((IC, OC, P, NCH), np.float32)
    for Cpp in range(NCH):
        gain_zc[:, :, :, S8[0][Cpp]] += gain_y8p[:, :, :, Cpp]
        gain_zc[:, :, :, S8[1][Cpp]] += gain_y8p[:, :, :, Cpp] * np.abs(
            r8[:, :, :, Cpp]
        )
    score = np.zeros((IC, OC, 4, 2), np.float32)
    for k in range(4):
        for po in range(2):
            Ss = F8MAX / np.maximum(
                np.maximum(Emax[:, :, k], Omax[:, :, k, po]), _EPS
            )
            v = O[:, :, k, po] * Ss[:, :, None, :]
            delta = (_f8(v) - v) / Ss[:, :, None, :]
            dn = np.linalg.norm(delta, axis=2)          # [ic,oc,m]
            score[:, :, k, po] = (dn * gain_zc[:, :, :, 2 * k + po]).max(axis=2)
    # positions: oc = core*OCL + 2*og + oo
    sc = score.reshape(IC, NCORES, NOG, 2, 4, 2).transpose(1, 2, 0, 3, 4, 5)
    vote = sc.max(axis=0)                               # [og, ic, oo, k, po]
    thr = np.quantile(vote, 1.0 - FRAC_BF16)
    mask = vote >= thr
    # cap bf16 slots per (og,ic) granule at 5 so granule sizes (and hence
    # per-iteration DMA arrival times) stay roughly uniform
    for og in range(NOG):
        for ic in range(IC):
            g = mask[og, ic]
            n = int(g.sum())
            if n > 5:
                v = vote[og, ic].ravel()
                keep = np.argsort(v)[::-1][:5]
                g2 = np.zeros(16, bool)
                g2[keep] = True
                mask[og, ic] = g2.reshape(2, 4, 2)
    # force the first granule all-fp8 so the first weight transfer is small
    # and the pipeline starts earlier
    mask[0, 0] = False
    return mask


def _prep_host(x, twiddle, bias):
    perm = _bitrev(N)
    y = np.ascontiguousarray(x).reshape(IC, B, N)[:, :, perm]
    y_dev = np.ascontiguousarray(
        y.reshape(IC, B, NCH, P).transpose(0, 3, 2, 1)
    ).reshape(IC, P, NCH * B).astype(BF)

    tw = np.asarray(twiddle, dtype=np.float32)
    A = _compose_A7(tw)                    # [IC, OC, NCH, P(out), P(in)]
    r7, r8, v0, v1 = _stageB_coeffs(tw)    # each [IC, OC, P, NCH]

    lhsT_E = A[:, :, 0::2].transpose(0, 1, 2, 4, 3)          # [ic,oc,4,p_in,m]
    lhsT_O = A[:, :, 1::2].transpose(0, 1, 2, 4, 3)
    r7m = r7.transpose(0, 1, 3, 2)                            # [ic,oc,C',m]
    O = lhsT_O[:, :, :, None] * r7m.reshape(IC, OC, 4, 2, 1, P)  # [ic,oc,k,po,p,m]

    mask16 = _slot_scores(lhsT_E, O, r8, v0, v1)   # [og,ic,oo,k,po] True=bf16

    # ragged blob layout, shared across cores: per (og,ic):
    #   [E: 512 u] [fp8 O slots: 64 u each] [bf16 O slots: 128 u each]
    # (u = bf16 unit = 2 bytes)
    offs = np.zeros((NOG, IC), np.int64)
    lens = np.zeros((NOG, IC), np.int64)
    cur = 0
    slot_pos = {}
    for og in range(NOG):
        for ic in range(IC):
            offs[og, ic] = cur
            n8 = int((~mask16[og, ic]).sum())
            i8 = i16 = 0
            for oo in range(2):
                for k in range(4):
                    for po in range(2):
                        if mask16[og, ic, oo, k, po]:
                            slot_pos[(og, ic, oo, k, po)] = (False, 512 + n8 * 64 + i16 * 128)
                            i16 += 1
                        else:
                            slot_pos[(og, ic, oo, k, po)] = (True, 512 + i8 * 64)
                            i8 += 1
            lens[og, ic] = 512 + n8 * 64 + i16 * 128
            cur += lens[og, ic]
    WTOT = int(cur)
    layout = {"offs": offs, "lens": lens, "slot_pos": slot_pos, "wtot": WTOT}

    bias_np = np.asarray(bias, dtype=np.float32).reshape(OC, NCH, P)

    in_maps, emu_maps = [], []
    for core in range(NCORES):
        osl = slice(core * OCL, (core + 1) * OCL)
        Eg = lhsT_E[:, osl]                     # [ic,ocl,k,p,m]
        Og = O[:, osl]                          # [ic,ocl,k,po,p,m]
        Emax = np.abs(Eg).max(axis=3)           # [ic,ocl,k,m]
        Omax = np.abs(Og).max(axis=4)           # [ic,ocl,k,po,m]
        # group scale covers E plus any fp8 O slot of the group
        m_oc = np.zeros((IC, OCL, 4, 2), bool)  # True = bf16
        for og in range(NOG):
            for oo in range(2):
                m_oc[:, 2 * og + oo] = mask16[og, :, oo]
        incl = np.where(m_oc.transpose(0, 1, 2, 3)[:, :, :, :, None], 0.0, Omax)
        gm = np.maximum(Emax, incl.max(axis=3))
        S = (F8MAX / np.maximum(gm, _EPS)).astype(np.float32)  # [ic,ocl,k,m]

        wEq = _f8(Eg * S[:, :, :, None, :])
        wOs = Og * S[:, :, :, None, None, :]
        for k in range(4):
            for po in range(2):
                sel = ~m_oc[:, :, k, po]
                wq = _f8(wOs[:, :, k, po])
                wOs[:, :, k, po] = np.where(sel[:, :, None, None], wq, wOs[:, :, k, po])

        Sp = S.transpose(0, 1, 3, 2)            # [ic,ocl,p,k]
        r8f = r8[:, osl] * (Sp[:, :, :, _G0] / Sp[:, :, :, _G1])
        v0f = v0[:, osl] / Sp[:, :, :, _G0[S9[0]]]
        v1f = v1[:, osl] / Sp[:, :, :, _G0[S9[1]]]

        # pack w blob
        wb = np.zeros((P, WTOT), dtype=BF)
        wb_u8 = wb.view(np.uint8).reshape(P, WTOT * 2)
        for og in range(NOG):
            for ic in range(IC):
                base = int(offs[og, ic])
                # E: [oo,k] fp8, 128 cols each -> 512 units
                eb = np.ascontiguousarray(
                    wEq[ic, 2 * og : 2 * og + 2].transpose(2, 0, 1, 3)
                ).reshape(P, 1024).astype(F8)
                wb_u8[:, base * 2 : base * 2 + 1024] = eb.view(np.uint8)
                for oo in range(2):
                    for k in range(4):
                        for po in range(2):
                            isf8, pos = slot_pos[(og, ic, oo, k, po)]
                            sl = np.ascontiguousarray(wOs[ic, 2 * og + oo, k, po])
                            bo = (base + pos) * 2
                            if isf8:
                                wb_u8[:, bo : bo + 128] = sl.astype(F8).view(np.uint8)
                            else:
                                wb_u8[:, bo : bo + 256] = sl.astype(BF).view(np.uint8)

        # ycf blob: y 512 | r8 dup 64 | v01 dup 128 | r8 nodup 32 (pad 736)
        ycf = np.zeros((IC, P, YCF_LEN), dtype=BF)
        ycf[:, :, YCF_Y:YCF_R8] = y_dev
        r8d = np.repeat(
            r8f.transpose(0, 2, 1, 3)[..., None], DUP, axis=-1
        ).reshape(IC, P, OCL * NCH * DUP)
        ycf[:, :, YCF_R8:YCF_V01] = r8d.astype(BF)
        v01d = np.repeat(
            np.stack([v0f, v1f], axis=2).transpose(0, 3, 1, 2, 4)[..., None],
            DUP, axis=-1,
        ).reshape(IC, P, OCL * 2 * NCH * DUP)
        ycf[:, :, YCF_V01:YCF_R8ND] = v01d.astype(BF)
        r8nd = r8f.transpose(0, 2, 1, 3).reshape(IC, P, OCL * NCH)
        ycf[:, :, YCF_R8ND : YCF_R8ND + OCL * NCH] = r8nd.astype(BF)

        # misc blob f32 [P, 96]: bias 32 f32 | ident 128 bf16 (64 f32 units)
        misc = np.zeros((P, 96), dtype=np.float32)
        misc[:, :32] = np.ascontiguousarray(
            bias_np[osl].transpose(2, 0, 1)
        ).reshape(P, OCL * NCH)
        ident = np.eye(P, dtype=np.float32).astype(BF)
        misc.view(np.uint8)[:, 128:384] = ident.view(np.uint8)

        in_maps.append({"w": wb, "ycf": ycf, "misc": misc})
        emu_maps.append(
            {
                "y": y_dev,
                "wE": wEq,      # f32 values (exactly fp8)
                "wO": wOs,      # f32 values (fp8 or bf16-roundable)
                "r8": r8f,
                "v01": (v0f, v1f),
                "bias": misc[:, :32],
            }
        )
    return in_maps, emu_maps, layout


def _emulate_core(em):
    """Numpy emulation of the device program (exact op/rounding semantics)."""
    y = em["y"].reshape(IC, P, NCH, B)
    wE, wO = em["wE"], em["wO"]
    r8a = em["r8"]
    v0a, v1a = em["v01"]
    bias = em["bias"].reshape(P, OCL, NCH)
    out = np.zeros((OCL, P, NCH, B), dtype=np.float32)
    for o in range(OCL):
        acc = np.zeros((P, NCH, B), dtype=np.float32)
        for ic in range(IC):
            yv = y[ic].astype(np.float32)
            z = np.zeros((P, NCH, B), dtype=np.float32)
            for Cp in range(NCH):
                k, po = Cp >> 1, Cp & 1
                lE = wE[ic, o, k]
                lO = wO[ic, o, k, po].astype(BF).astype(np.float32)
                z[:, Cp] = lE.T @ yv[:, 2 * k] + lO.T @ yv[:, 2 * k + 1]
            zc = z.astype(BF).astype(np.float32)
            r8 = r8a[ic, o].astype(BF).astype(np.float32)
            v0 = v0a[ic, o].astype(BF).astype(np.float32)
            v1 = v1a[ic, o].astype(BF).astype(np.float32)
            tmp8 = (r8[:, :, None] * zc[:, S8[1]]).astype(BF).astype(np.float32)
            y8p = (zc[:, S8[0]] + tmp8).astype(BF).astype(np.float32)
            t9a = (v0[:, :, None] * y8p[:, S9[0]]).astype(BF).astype(np.float32)
            t9b = (v1[:, :, None] * y8p[:, S9[1]]).astype(BF).astype(np.float32)
            acc += t9a + t9b
        out[o] = acc + bias[:, o, :, None]
    return out.reshape(OCL, P, NCH * B)


_LAST_RESULTS = {"exec_time_ns": None}


def kernel(x, twiddle, bias, _trace=False, _emulate=False):
    in_maps, emu_maps, layout = _prep_host(
        np.asarray(x), np.asarray(twiddle), np.asarray(bias)
    )
    if _emulate:
        outs = [_emulate_core(em) for em in emu_maps]
    else:
        from concourse.bass_utils import run_bass_kernel_spmd

        nc = _build_program(layout)
        res = run_bass_kernel_spmd(nc, in_maps, list(range(NCORES)), trace=_trace)
        _LAST_RESULTS["exec_time_ns"] = res.exec_time_ns
        _LAST_RESULTS["mean_exec_time_ns"] = res.mean_exec_time_ns
        outs = [r["o"] for r in res.results]
    full = np.concatenate(
        [
            np.asarray(o, dtype=np.float32)
            .reshape(OCL, P, NCH, B)
            .transpose(0, 3, 2, 1)
            .reshape(OCL, B, N)
            for o in outs
        ],
        axis=0,
    )
    return np.ascontiguousarray(full).reshape(B, OC, H, W).astype(np.float32)


def _build_program(layout):
    import concourse.bacc as bacc
    import concourse.mybir as mybir
    from concourse.tile import TileContext

    bf = mybir.dt.bfloat16
    f8 = mybir.dt.float8e4
    f32 = mybir.dt.float32
    MULT, ADD = mybir.AluOpType.mult, mybir.AluOpType.add
    G = B // DUP
    offs, lens, slot_pos = layout["offs"], layout["lens"], layout["slot_pos"]
    WTOT = layout["wtot"]
    WMAX = int(lens.max())

    nc = bacc.Bacc(None, target_bir_lowering=False)
    w_d = nc.dram_tensor("w", (P, WTOT), bf, kind="ExternalInput")
    ycf_d = nc.dram_tensor("ycf", (IC, P, YCF_LEN), bf, kind="ExternalInput")
    misc_d = nc.dram_tensor("misc", (P, 96), f32, kind="ExternalInput")
    o_d = nc.dram_tensor("o", (OCL, P, NCH * B), f32, kind="ExternalOutput")

    NB = NCH * B  # 512

    with TileContext(nc) as tc:
        with (
            tc.tile_pool(name="persist", bufs=1) as persist,
            tc.tile_pool(name="wpool", bufs=11) as wpool,
            tc.tile_pool(name="zcpool", bufs=3) as zcpool,
            tc.tile_pool(name="sb1", bufs=4) as sb1,
            tc.tile_pool(name="sb2", bufs=3) as sb2,
            tc.tile_pool(name="outp", bufs=1) as outp,
            tc.tile_pool(name="psz", bufs=4, space="PSUM") as psz,
            tc.tile_pool(name="psacc", bufs=OCL, space="PSUM") as psacc,
        ):
            misct = persist.tile([P, 96], f32, tag="misc")
            nc.scalar.dma_start(out=misct[:], in_=misc_d[:])
            biast = misct[:, 0:32]
            idt = misct[:, 32:96].bitcast(bf)  # [P, 128]

            accs = [psacc.tile([P, NB], f32, tag="acc", name=f"acc{i}") for i in range(OCL)]
            ycfts = [None] * IC

            # PE warm-up: dummy matmuls into accs[0] while the first weight
            # granule streams in, so the HAM clock-gate releases (1.2->2.4GHz)
            # before the real stage-A matmuls arrive.  A memset scratch tile
            # (no DMA dependency) lets them start as soon as the engines come
            # up (~6us) instead of waiting for the misc transfer (~11.6us
            # measured).  The first identity flush writes accs[0] with
            # start=True, which resets PSUM and discards this garbage.
            warmt = persist.tile([P, P], bf, tag="warm")
            nc.vector.memset(warmt[:], 0.0)
            for _ in range(28):
                nc.tensor.matmul(
                    accs[0][:, 0:P],
                    warmt[:],
                    warmt[:],
                    start=True,
                    stop=True,
                    skip_group_check=True,
                )

            DELAY = 2
            deferred = []

            def flush_one():
                t9_, ic_, og_ = deferred.pop(0)
                for oo_ in range(2):
                    o_ = 2 * og_ + oo_
                    for e_ in range(2):
                        nc.tensor.matmul(
                            accs[o_][:],
                            idt,
                            t9_[:, (2 * oo_ + e_) * NB : (2 * oo_ + e_ + 1) * NB],
                            start=(ic_ == 0 and e_ == 0),
                            stop=(ic_ == IC - 1 and e_ == 1),
                        )

            ots = [None] * OCL

            def bias_add(og_):
                for oo in range(2):
                    o = 2 * og_ + oo
                    ots[o] = outp.tile([P, NB], f32, tag=f"out{o}", name=f"ot{o}")
                    nc.vector.tensor_tensor(
                        ots[o][:].rearrange("p (c b) -> p c b", c=NCH),
                        accs[o][:].rearrange("p (c b) -> p c b", c=NCH),
                        biast[:, o * NCH : (o + 1) * NCH]
                        .unsqueeze(2)
                        .broadcast_to((P, NCH, B)),
                        ADD,
                    )

            def store_out(og_):
                # og0 mid-run: scalar queue (sync still streams weights);
                # og1 at the end: sync queue is idle by then
                eng = nc.scalar if og_ == 0 else nc.sync
                for oo in range(2):
                    o = 2 * og_ + oo
                    eng.dma_start(out=o_d[o], in_=ots[o][:])

            for og in range(NOG):
                for ic in range(IC):
                    if og == 0:
                        ycfts[ic] = persist.tile(
                            [P, YCF_LEN], bf, tag=f"ycf{ic}", name=f"ycf{ic}"
                        )
                        nc.gpsimd.dma_start(out=ycfts[ic][:], in_=ycf_d[ic])
                    ycft = ycfts[ic]
                    yt = ycft[:, YCF_Y:YCF_R8]

                    L = int(lens[og, ic])
                    base = int(offs[og, ic])
                    wt = wpool.tile([P, WMAX], bf, name="wt")
                    nc.sync.dma_start(out=wt[:, 0:L], in_=w_d[:, base : base + L])
                    wE8 = wt[:, 0:512].bitcast(f8)   # [P, 1024]

                    zc2 = zcpool.tile([P, 2 * NB], bf)
                    for oo in range(2):
                        z = psz.tile([P, NB], f32, tag="z", name=f"z{oo}")
                        for k in range(4):
                            nc.tensor.matmul(
                                z[:, (2 * k) * B : (2 * k + 2) * B],
                                wE8[:, (oo * 4 + k) * P : (oo * 4 + k + 1) * P],
                                yt[:, (2 * k) * B : (2 * k + 1) * B]
                                .unsqueeze(1)
                                .broadcast_to((P, 2, B)),
                                start=True,
                                stop=False,
                                skip_group_check=True,
                            )
                            for po in range(2):
                                isf8, pos = slot_pos[(og, ic, oo, k, po)]
                                if isf8:
                                    lhsT = wt[:, pos : pos + 64].bitcast(f8)
                                else:
                                    lhsT = wt[:, pos : pos + P]
                                nc.tensor.matmul(
                                    z[:, (2 * k + po) * B : (2 * k + po + 1) * B],
                                    lhsT,
                                    yt[:, (2 * k + 1) * B : (2 * k + 2) * B],
                                    start=False,
                                    stop=True,
                                    skip_group_check=True,
                                )
                        nc.scalar.activation(
                            zc2[:, oo * NB : (oo + 1) * NB],
                            z[:],
                            mybir.ActivationFunctionType.Copy,
                        )
                    # ---- L8/L9 elementwise ----
                    t8t = sb1.tile([P, 2 * NB], bf, tag="t8")
                    if GPS_T8:
                        # oc-half 0 on GpSimd with 4D APs (non-dup r8):
                        # out/in views: (k t n b); src chunk s1 = 4k+2+n
                        o0 = 2 * og
                        t8g = t8t[:, 0:NB].rearrange(
                            "p (k t n b) -> p k t n b", k=2, t=2, n=2
                        )
                        zcg = (
                            zc2[:, 0:NB]
                            .rearrange("p (k q n b) -> p k q n b", k=2, q=2, n=2)[
                                :, :, 1:2
                            ]
                            .broadcast_to((P, 2, 2, 2, B))
                        )
                        r8g = (
                            ycft[:, YCF_R8ND + o0 * NCH : YCF_R8ND + (o0 + 1) * NCH]
                            .rearrange("p (k t n) -> p k t n", k=2, t=2)
                            .unsqueeze(4)
                            .broadcast_to((P, 2, 2, 2, B))
                        )
                        nc.gpsimd.tensor_tensor(t8g, zcg, r8g, MULT)
                        # oc-half 1 on DVE (dup-2 packed)
                        sh8h = (P, 1, 2, 2, 2, G, DUP)
                        zq1 = zc2[:, NB : 2 * NB].rearrange(
                            "p (k q n g d) -> p k q n g d", k=2, q=2, n=2, d=DUP
                        )
                        r8o1 = (
                            ycft[
                                :,
                                YCF_R8 + (o0 + 1) * NCH * DUP : YCF_R8
                                + (o0 + 2) * NCH * DUP,
                            ]
                            .rearrange("p (k t n d) -> p k t n d", k=2, t=2, d=DUP)
                            .unsqueeze(4)
                            .broadcast_to((P, 2, 2, 2, G, DUP))
                        )
                        t8v1 = t8t[:, NB : 2 * NB].rearrange(
                            "p (k t n g d) -> p k t n g d", k=2, t=2, n=2, d=DUP
                        )
                        nc.vector.tensor_tensor(
                            t8v1,
                            zq1[:, :, 1:2, :, :, :].broadcast_to((P, 2, 2, 2, G, DUP)),
                            r8o1,
                            MULT,
                        )
                    else:
                        zq = zc2[:].rearrange(
                            "p (O k q n g d) -> p O k q n g d", O=2, k=2, q=2, n=2, d=DUP
                        )
                        sh8 = (P, 2, 2, 2, 2, G, DUP)
                        r8o = (
                            ycft[:, YCF_R8 + og * 2 * NCH * DUP : YCF_R8 + (og + 1) * 2 * NCH * DUP]
                            .rearrange("p (O k t n d) -> p O k t n d", O=2, k=2, t=2, d=DUP)
                            .unsqueeze(5)
                            .broadcast_to(sh8)
                        )
                        t8v = t8t[:].rearrange(
                            "p (O k t n g d) -> p O k t n g d", O=2, k=2, t=2, n=2, d=DUP
                        )
                        nc.vector.tensor_tensor(
                            t8v, zq[:, :, :, 1:2, :, :, :].broadcast_to(sh8), r8o, MULT
                        )
                    zq = zc2[:].rearrange(
                        "p (O k q n g d) -> p O k q n g d", O=2, k=2, q=2, n=2, d=DUP
                    )
                    sh8 = (P, 2, 2, 2, 2, G, DUP)
                    y8t = sb2.tile([P, 2 * NB], bf, tag="y8")
                    y8v = y8t[:].rearrange(
                        "p (O k t n g d) -> p O k t n g d", O=2, k=2, t=2, n=2, d=DUP
                    )
                    t8v = t8t[:].rearrange(
                        "p (O k t n g d) -> p O k t n g d", O=2, k=2, t=2, n=2, d=DUP
                    )
                    nc.vector.tensor_tensor(
                        y8v, zq[:, :, :, 0:1, :, :, :].broadcast_to(sh8), t8v, ADD
                    )
                    y8q = y8t[:].rearrange(
                        "p (O q m g d) -> p O q m g d", O=2, q=2, m=4, d=DUP
                    )
                    sh9 = (P, 2, 2, 2, 4, G, DUP)
                    v01o = (
                        ycft[:, YCF_V01 + og * 4 * NCH * DUP : YCF_V01 + (og + 1) * 4 * NCH * DUP]
                        .rearrange("p (O e t m d) -> p O e t m d", O=2, e=2, t=2, d=DUP)
                        .unsqueeze(5)
                        .broadcast_to(sh9)
                    )
                    t9 = sb1.tile([P, 4 * NB], bf, tag="t9")
                    t9v = t9[:].rearrange(
                        "p (O e t m g d) -> p O e t m g d", O=2, e=2, t=2, m=4, d=DUP
                    )
                    nc.vector.tensor_tensor(
                        t9v, y8q[:].unsqueeze(3).broadcast_to(sh9), v01o, MULT
                    )
                    deferred.append((t9, ic, og))
                    # taper the deferral near the end so the final flushes
                    # don't stack up serially after the last t9
                    dly = 1 if (og == 1 and ic >= IC - 3) else DELAY
                    while len(deferred) > dly:
                        flush_one()
                    # og0's bias happens mid-og1 so it doesn't stall the DVE
                    # stream at the og boundary; its out-DMA waits until the
                    # weight stream has drained (out transfers displace
                    # incoming w granules otherwise)
                    if og == 1 and ic == 3:
                        bias_add(0)
                    if og == 1 and ic == 12:
                        store_out(0)
            while deferred:
                flush_one()
            bias_add(1)
            store_out(1)
    nc.finalize()
    return nc
